# revision 51
# baseline (speedup 1.0000x reference)
"""GNN message-passing layer (out = relu(segment_sum(vals * (xW)[src] by dst)))
on 8 Trainium2 NeuronCores.

Strategy (1D graph partitioning, per sharding hint):
- dst nodes are permuted into 8*BLOCKS blocks of 128, degree-balanced jointly
  on lo/hi source halves; core m owns blocks [m*BLOCKS, (m+1)*BLOCKS) and
  produces those output rows.
- Every core computes the full dense transform h = x @ W (replicated, bf16)
  into two DRAM tensors h_lo/h_hi (row split at n_pad/2 so gather indices fit
  int16), via bf16 matmuls.
- Per group of dst blocks: two big SWDGE dma_gather instructions (one per h
  half) fetch all source rows h[src] into SBUF in one descriptor batch each,
  amortizing the ~1us fixed SWDGE overhead that dominated the per-chunk
  indirect-DMA baseline. A value-scaled one-hot matrix P (built on DVE from
  iota/is_equal, all bf16) is matmul'd against the messages, accumulating all
  chunks of a block into one PSUM tile; relu + store.
- Padding slots carry val = 0 and gather row 0, so they contribute nothing.
"""
import math
from contextlib import ExitStack

import numpy as np

import concourse.bass as bass
import concourse.mybir as mybir
import concourse.tile as tile
from concourse.bass_utils import run_bass_kernel_spmd
from concourse.vector_clock import ScopedClock

# --- workaround: this walrus build rejects >1 sync wait per instruction
# ("Too many sync wait commands"). Tile's kernel-tail drain carries one wait
# per active sem lane; give it the same NOP-splitting treatment as everything
# else via a post-schedule legalization pass over all basic blocks. ---
_MAX_WAITS = 1


def _patched_drain_and_barrier(self, tick_clock, wait_clock):
    drain_inst = self.nc.sync.drain()
    wait_clock.add_sem_waits(
        drain_inst.ins, ScopedClock({None: tick_clock.global_clock})
    )
    self.nc.all_engine_barrier()
    popped = self.nc._tile_sem_poison_stack.pop()
    assert popped is self._sem_poison
    self.nc.clear_and_free_semaphores(list(self.sems.allocated().values()))
    self.nc.all_engine_barrier()


tile.TileContext._drain_and_barrier = _patched_drain_and_barrier


def _legalize_sync_waits(nc):
    """Split instructions carrying >_MAX_WAITS sem waits: excess waits move to
    same-engine NOPs inserted immediately before the instruction."""
    n_split = 0
    for f in nc.m.functions:
        for bb in f.blocks:
            out = []
            changed = False
            for ins in bb.instructions:
                si = ins.sync_info
                waits = list(si.on_wait) if si and si.on_wait else []
                if len(waits) > _MAX_WAITS:
                    changed = True
                    n_split += 1
                    for i in range(_MAX_WAITS, len(waits), _MAX_WAITS):
                        nop = mybir.InstNoOp(
                            name=nc.get_next_instruction_name(), ins=[], outs=[]
                        )
                        nop.engine = ins.engine
                        nop.sync_info = mybir.SyncInfo(
                            on_wait=waits[i : i + _MAX_WAITS], on_update=[]
                        )
                        nc.register_instruction(nop, overwrite=True)
                        out.append(nop)
                    si.on_wait = waits[:_MAX_WAITS]
                out.append(ins)
            if changed:
                bb.instructions = out
    return n_split


N_CORES = 8
P = 128
N_SWDGE_QUEUES = 4
DMA_SCRATCH = 16384  # HW SWDGE ring is fixed at 1024 descriptors
MAX_GATHER_CHUNKS = 32  # 32*128 = 4096 idxs per dma_gather (single_packet=False)


def _assign_gather_queues(nc):
    """Spread dma_gathers over the SWDGE queues so desc-gen/transfer of
    consecutive ring-fulls overlap. The queue must be a pure function of the
    instruction's Tile-assigned DMASW sem lane (assigned in *scheduled*
    order), else a sem lane receives updates from two queues, which HW
    forbids."""
    from concourse.tile_scheduler import PROC_NAME_TO_IDX

    base = PROC_NAME_TO_IDX["DMASW0"]
    n = 0
    for f in nc.m.functions:
        for bb in f.blocks:
            for ins in bb.instructions:
                if isinstance(ins, mybir.InstDMAGatherAnt):
                    lane = ins.bass_scheduled_proc - base
                    assert 0 <= lane < 8, (lane, ins.name)
                    ins.queue_num = lane % N_SWDGE_QUEUES
                    n += 1
    return n


def build_nc(lo_blk, hi_blk, d_in, d_out, blocks, c_lo, c_hi, groups):
    """One SPMD program with a sharded dense phase.

    Each core computes h = xW for its own shard of lo_blk+hi_blk node blocks,
    then two AllGathers build the replicated h_lo/h_hi tensors (each half
    < 32768 rows so gather idxs fit int16). lo gathers overlap the hi
    compute + AllGather.
    """
    f32 = mybir.dt.float32
    bf16 = mybir.dt.bfloat16
    i16 = mybir.dt.int16
    KD = d_in // P
    NB = lo_blk + hi_blk
    r_lo, r_hi = lo_blk * P, hi_blk * P
    assert r_lo * N_CORES < 32768 and r_hi * N_CORES < 32768
    nch = c_lo + c_hi
    ni_lo = [g * c_lo * P for g in groups]  # gather idx counts per group
    ni_hi = [g * c_hi * P for g in groups]
    ni_cols = sum(n // 16 for n in ni_lo + ni_hi)

    nc = bass.Bass(
        num_swdge_queues=N_SWDGE_QUEUES,
        dynamic_dma_scratch_size=DMA_SCRATCH,
        num_devices=N_CORES,
    )
    xT = nc.declare_dram_parameter("xT", [d_in, NB * P], bf16, isOutput=False)
    Wp = nc.declare_dram_parameter("W", [d_in, d_out], bf16, isOutput=False)
    dstp = nc.declare_dram_parameter("dstv", [P, blocks * nch], bf16, isOutput=False)
    valp = nc.declare_dram_parameter("valv", [P, blocks * nch], bf16, isOutput=False)
    idxp = nc.declare_dram_parameter("idx", [P, ni_cols], i16, isOutput=False)
    iotap = nc.declare_dram_parameter("iota", [P, nch * P], bf16, isOutput=False)
    outp = nc.declare_dram_parameter("out", [blocks * P, d_out], f32, isOutput=True)
    h_my_lo = nc.dram_tensor("h_my_lo", [r_lo, d_out], bf16)
    h_my_hi = nc.dram_tensor("h_my_hi", [r_hi, d_out], bf16)
    h_lo = nc.dram_tensor("h_lo", [r_lo * N_CORES, d_out], bf16, addr_space="Shared")
    h_hi = nc.dram_tensor("h_hi", [r_hi * N_CORES, d_out], bf16, addr_space="Shared")
    ALL = [list(range(N_CORES))]

    with tile.TileContext(nc) as tc:
        with ExitStack() as ctx:
            wpool = ctx.enter_context(tc.tile_pool(name="w", bufs=1))
            epool = ctx.enter_context(tc.tile_pool(name="edges", bufs=1))
            xpool = ctx.enter_context(tc.tile_pool(name="xs", bufs=2))
            hpool = ctx.enter_context(tc.tile_pool(name="hs", bufs=2))
            ps1 = ctx.enter_context(tc.tile_pool(name="ps1", bufs=4, space="PSUM"))
            mpool = ctx.enter_context(tc.tile_pool(name="msgs", bufs=3))
            ppool = ctx.enter_context(tc.tile_pool(name="onehot", bufs=4))
            ps2 = ctx.enter_context(tc.tile_pool(name="ps2", bufs=4, space="PSUM"))
            opool = ctx.enter_context(tc.tile_pool(name="osb", bufs=2))

            # --- constants / per-core edge data, loaded once ---
            w_t = wpool.tile([P, KD * d_out], bf16)
            for k in range(KD):
                nc.sync.dma_start(
                    w_t[:, k * d_out : (k + 1) * d_out], Wp[k * P : (k + 1) * P, :]
                )
            dst_t = epool.tile([P, blocks * nch], bf16)
            val_t = epool.tile([P, blocks * nch], bf16)
            idx_t = epool.tile([P, ni_cols], i16)
            nc.sync.dma_start(dst_t[:], dstp[:])
            nc.sync.dma_start(val_t[:], valp[:])
            nc.sync.dma_start(idx_t[:], idxp[:])
            # iota comes from the host (it's just lane indices) so gpsimd has
            # no 'standard'-library work: load_library(mlp) runs at t=0 and
            # the AllGather trigger isn't stuck behind a slow ucode reload.
            iota_t = epool.tile([P, nch * P], bf16)
            nc.sync.dma_start(iota_t[:], iotap[:])
            from concourse import library_config

            nc.gpsimd.load_library(library_config.mlp)

            # --- phase 1: h = x @ W over this core's shard, then AllGather ---
            for htgt, hfull, boff, nb in (
                (h_my_lo, h_lo, 0, lo_blk),
                (h_my_hi, h_hi, lo_blk, hi_blk),
            ):
                for s0 in range(0, nb, 8):
                    sb = min(8, nb - s0)
                    sn = sb * P
                    xs = xpool.tile([P, KD * 8 * P], bf16, tag="xs")
                    for k in range(KD):
                        nc.sync.dma_start(
                            xs[:, k * sn : (k + 1) * sn],
                            xT[k * P : (k + 1) * P, (boff + s0) * P :
                               (boff + s0) * P + sn],
                        )
                    hs = hpool.tile([P, 8 * d_out], bf16, tag="hs")
                    for j in range(sb):
                        pt = ps1.tile([P, d_out], f32)
                        for k in range(KD):
                            nc.tensor.matmul(
                                pt[:],
                                lhsT=xs[:, k * sn + j * P : k * sn + (j + 1) * P],
                                rhs=w_t[:, k * d_out : (k + 1) * d_out],
                                start=(k == 0),
                                stop=(k == KD - 1),
                            )
                        nc.vector.tensor_copy(
                            hs[:, j * d_out : (j + 1) * d_out], pt[:]
                        )
                    hd = htgt[s0 * P : s0 * P + sn, :].rearrange(
                        "(nb p) f -> p nb f", p=P
                    )
                    nc.sync.dma_start(
                        hd, hs[:, : sb * d_out].rearrange("p (nb f) -> p nb f", nb=sb)
                    )
                nc.gpsimd.collective_compute(
                    "AllGather",
                    mybir.AluOpType.bypass,
                    ALL,
                    [htgt[:]],
                    [hfull[:]],
                )

            # --- phase 2: grouped SWDGE gathers + one-hot scatter matmuls ---
            ni_regs = {}  # Pool GPRs are scarce; one per distinct num_idxs

            def ni_reg(v):
                if v not in ni_regs:
                    ni_regs[v] = nc.gpsimd.to_reg(v)
                return ni_regs[v]

            def gather_windows(mtile, h_src, nchunks, col0):
                """Fetch nchunks*P rows into mtile via <=MAX_GATHER_CHUNKS-chunk
                dma_gather instructions (ring holds 1024 descriptors)."""
                for w0 in range(0, nchunks, MAX_GATHER_CHUNKS):
                    wc = min(MAX_GATHER_CHUNKS, nchunks - w0)
                    ni = wc * P
                    nc.gpsimd.dma_gather(
                        out_ap=mtile[:, w0 * d_out : (w0 + wc) * d_out].rearrange(
                            "p (c f) -> p c f", f=d_out
                        ),
                        in_ap=h_src[:],
                        idxs_ap=idx_t[:, col0 + w0 * 8 : col0 + w0 * 8 + ni // 16],
                        num_idxs=ni,
                        num_idxs_reg=ni_reg(ni),
                        elem_size=d_out,
                        queue_num=0,
                        single_packet=False,
                    )

            b0 = 0
            for gi, g in enumerate(groups):
                mlo = mpool.tile([P, g * c_lo * d_out], bf16, tag="mlo")
                mhi = mpool.tile([P, g * c_hi * d_out], bf16, tag="mhi")
                col_lo = sum(n // 16 for n in ni_lo[:gi] + ni_hi[:gi])
                col_hi = col_lo + ni_lo[gi] // 16
                gather_windows(mlo, h_lo, g * c_lo, col_lo)
                gather_windows(mhi, h_hi, g * c_hi, col_hi)
                for bi in range(g):
                    b = b0 + bi
                    pt3 = ppool.tile([P, nch * P], bf16, tag="P")
                    iota3 = bass.AP(
                        iota_t[:].tensor, iota_t[:].offset,
                        [iota_t[:].ap[0], [P, nch], [1, P]],
                    )
                    p3 = bass.AP(
                        pt3[:].tensor, pt3[:].offset, [pt3[:].ap[0], [P, nch], [1, P]]
                    )
                    dstb = dst_t[:, b * nch : (b + 1) * nch]
                    valb = val_t[:, b * nch : (b + 1) * nch]
                    dst_bc = bass.AP(dstb.tensor, dstb.offset, dstb.ap + [[0, P]])
                    val_bc = bass.AP(valb.tensor, valb.offset, valb.ap + [[0, P]])
                    nc.vector.tensor_tensor(
                        out=p3, in0=iota3, in1=dst_bc, op=mybir.AluOpType.is_equal
                    )
                    nc.vector.tensor_tensor(
                        out=p3, in0=p3, in1=val_bc, op=mybir.AluOpType.mult
                    )
                    acc = ps2.tile([P, d_out], f32)
                    for c in range(nch):
                        src_tile, cc = (mlo, bi * c_lo + c) if c < c_lo else (
                            mhi, bi * c_hi + (c - c_lo)
                        )
                        nc.tensor.matmul(
                            acc[:],
                            lhsT=pt3[:, c * P : (c + 1) * P],
                            rhs=src_tile[:, cc * d_out : (cc + 1) * d_out],
                            start=(c == 0),
                            stop=(c == nch - 1),
                        )
                    ot = opool.tile([P, d_out], f32)
                    nc.scalar.activation(
                        ot[:], acc[:], mybir.ActivationFunctionType.Relu
                    )
                    nc.sync.dma_start(outp[b * P : (b + 1) * P, :], ot[:])
                b0 += g
    # Raw bass skips Bacc's post-passes; the extended-inst ISA subclasses
    # (DMAGatherAnt, LOAD_LIB) need real LOAD_LIBs + .instr bytes or walrus
    # fails with "ISA wrong length".
    import bass_rust as _bass_rust
    from concourse.library_config import all_libraries, standard

    inst_type_to_lib_mask = {}
    for lib in all_libraries:
        for inst_type in lib.instructions:
            inst_type_to_lib_mask[inst_type] = inst_type_to_lib_mask.get(
                inst_type, 0
            ) | (1 << lib.index)
    _assign_gather_queues(nc)
    _bass_rust.insert_library_loads(
        nc, inst_type_to_lib_mask, len(all_libraries), standard.index
    )
    _legalize_sync_waits(nc)
    mybir.codegen_inst_isa_subclasses(nc)
    return nc


def _pack_edges(
    edge_src, edge_dst, edge_vals, n_nodes, n_pad, blocks, group_sz, lo_blk, hi_blk
):
    """Permute dst nodes into jointly (lo/hi) degree-balanced blocks of 128;
    pack edges into per-core dst/val arrays plus int16 gather-index arrays.

    The dense phase is sharded: core m computes h for global x rows
    [m*shard, (m+1)*shard); the first lo_blk*128 rows of each shard are
    AllGathered into h_lo (rank-major), the rest into h_hi. Gather indices
    are relative to those gathered tensors."""
    import heapq

    shard = lo_blk * P + hi_blk * P
    r_lo, r_hi = lo_blk * P, hi_blk * P
    s0_frac = r_lo / shard
    total_blocks = N_CORES * blocks
    src_rank = edge_src // shard
    src_loc = edge_src % shard
    is_lo = src_loc < r_lo
    src_rel = np.where(
        is_lo, src_rank * r_lo + src_loc, src_rank * r_hi + (src_loc - r_lo)
    ).astype(np.int32)
    deg_lo = np.bincount(edge_dst[is_lo], minlength=n_nodes).astype(np.int64)
    deg_hi = np.bincount(edge_dst[~is_lo], minlength=n_nodes).astype(np.int64)
    deg = deg_lo + deg_hi
    order = np.argsort(-deg, kind="stable")
    heap = [(0.0, b) for b in range(total_blocks)]
    heapq.heapify(heap)
    counts = np.zeros(total_blocks, np.int32)
    loads_lo = np.zeros(total_blocks, np.int64)
    loads_hi = np.zeros(total_blocks, np.int64)
    block_of = np.empty(n_nodes, np.int32)
    slot_of = np.empty(n_nodes, np.int32)
    wl, wh = 1.0 / max(s0_frac, 1e-6), 1.0 / max(1.0 - s0_frac, 1e-6)
    for node in order:
        while True:
            load, b = heapq.heappop(heap)
            if counts[b] < P:
                break
        block_of[node] = b
        slot_of[node] = counts[b]
        counts[b] += 1
        loads_lo[b] += deg_lo[node]
        loads_hi[b] += deg_hi[node]
        if counts[b] < P:
            heapq.heappush(heap, (max(loads_lo[b] * wl, loads_hi[b] * wh), b))
    c_lo = max(1, math.ceil(loads_lo.max() / P))
    c_hi = max(1, math.ceil(loads_hi.max() / P))

    # order edges by (block, hi-half) and compute positions within each side
    eb = block_of[edge_dst]
    side_key = eb * 2 + (~is_lo).astype(np.int64)
    eorder = np.argsort(side_key, kind="stable")
    sk_sorted = side_key[eorder]
    ssizes = np.bincount(sk_sorted, minlength=total_blocks * 2)
    soffs = np.concatenate([[0], np.cumsum(ssizes)[:-1]])
    pos = np.arange(len(edge_src)) - soffs[sk_sorted]

    nch = c_lo + c_hi
    src_a = np.zeros((total_blocks, nch, P), np.int32)  # local row in segment
    dst_a = np.zeros((total_blocks, nch, P), np.float32)
    val_a = np.zeros((total_blocks, nch, P), np.float32)
    e_hi = (sk_sorted & 1).astype(bool)
    ch = (pos >> 7) + np.where(e_hi, c_lo, 0)
    lane = pos & 127
    src_a[sk_sorted >> 1, ch, lane] = src_rel[eorder]
    dst_a[sk_sorted >> 1, ch, lane] = slot_of[edge_dst[eorder]]
    val_a[sk_sorted >> 1, ch, lane] = edge_vals[eorder]

    # group structure (identical across cores)
    groups = []
    rem = blocks
    while rem > 0:
        g = min(group_sz, rem)
        groups.append(g)
        rem -= g

    import ml_dtypes

    dst_c, val_c, idx_c = [], [], []
    for m in range(N_CORES):
        blk = slice(m * blocks, (m + 1) * blocks)
        dst_c.append(
            np.ascontiguousarray(
                dst_a[blk].transpose(2, 0, 1).reshape(P, -1)
            ).astype(ml_dtypes.bfloat16)
        )
        val_c.append(
            np.ascontiguousarray(
                val_a[blk].transpose(2, 0, 1).reshape(P, -1)
            ).astype(ml_dtypes.bfloat16)
        )
        # gather idx stream: per group, lo idxs then hi idxs; within a side,
        # slot k = (block_in_group * c_side + chunk) * 128 + lane, laid out
        # int16 at [k % 16, k // 16] and replicated across the 8 lane-groups.
        cols = []
        b0 = m * blocks
        for g in groups:
            sa = src_a[b0 : b0 + g]
            lo = sa[:, :c_lo, :].reshape(-1)  # [g*c_lo*128] in slot order
            hi = sa[:, c_lo:, :].reshape(-1)
            cols.append(lo)
            cols.append(hi)
            b0 += g
        flat = np.concatenate(cols)
        wrapped = flat.reshape(-1, 16).T.astype(np.int16)  # [16, cols]
        idx_c.append(np.ascontiguousarray(np.tile(wrapped, (8, 1))))
    return dst_c, val_c, idx_c, block_of, slot_of, c_lo, c_hi, groups


def _build(x, W, edge_vals, edge_src, edge_dst, blocks=None, group_sz=4):
    """Pack inputs + build the SPMD program. Returns (nc, in_maps, gidx)."""
    import ml_dtypes

    n_nodes, d_in = x.shape
    d_out = W.shape[1]
    if blocks is None:
        blocks = math.ceil(n_nodes / (N_CORES * P))
    # pad node count to a multiple of 8*128 so shards are uniform
    n_pad = math.ceil(n_nodes / (N_CORES * P)) * N_CORES * P
    shard_blk = n_pad // (N_CORES * P)
    lo_blk = max(1, int(round(shard_blk * 0.45)))
    hi_blk = shard_blk - lo_blk
    dst_c, val_c, idx_c, block_of, slot_of, c_lo, c_hi, groups = _pack_edges(
        edge_src, edge_dst, edge_vals, n_nodes, n_pad, blocks, group_sz,
        lo_blk, hi_blk,
    )
    Wb = W.astype(ml_dtypes.bfloat16)

    # per-core xT: this core's contiguous shard of x rows
    shard = shard_blk * P
    xp = np.zeros((n_pad, d_in), np.float32)
    xp[:n_nodes] = x
    xT_c = [
        np.ascontiguousarray(xp[m * shard : (m + 1) * shard].T).astype(
            ml_dtypes.bfloat16
        )
        for m in range(N_CORES)
    ]

    nc = build_nc(lo_blk, hi_blk, d_in, d_out, blocks, c_lo, c_hi, groups)
    nch = c_lo + c_hi
    iota_np = np.tile(
        np.arange(P, dtype=np.float32), (P, nch)
    ).astype(ml_dtypes.bfloat16)
    in_maps = [
        {
            "xT": xT_c[m], "W": Wb, "dstv": dst_c[m], "valv": val_c[m],
            "idx": idx_c[m], "iota": iota_np,
        }
        for m in range(N_CORES)
    ]
    npc = blocks * P
    gidx = (block_of // blocks) * npc + (block_of % blocks) * P + slot_of
    meta = dict(lo_blk=lo_blk, hi_blk=hi_blk, c_lo=c_lo, c_hi=c_hi, groups=groups)
    return nc, in_maps, gidx, meta


def _run(x, W, edge_vals, edge_src, edge_dst, blocks=None, trace=False, group_sz=4):
    nc, in_maps, gidx, _ = _build(
        x, W, edge_vals, edge_src, edge_dst, blocks=blocks, group_sz=group_sz
    )
    res = run_bass_kernel_spmd(nc, in_maps, list(range(N_CORES)), trace=trace)
    stacked = np.concatenate([res.results[m]["out"] for m in range(N_CORES)], axis=0)
    out = stacked[gidx]
    return out, res


def kernel(x, W, edge_vals, edge_src, edge_dst):
    x = np.asarray(x, np.float32)
    W = np.asarray(W, np.float32)
    edge_vals = np.asarray(edge_vals, np.float32)
    edge_src = np.asarray(edge_src).astype(np.int64)
    edge_dst = np.asarray(edge_dst).astype(np.int64)
    out, _ = _run(x, W, edge_vals, edge_src, edge_dst)
    return out.astype(np.float32)
